# revision 1
# baseline (speedup 1.0000x reference)
"""Trainium2 Bass kernel for AdaptiveFeatureSelector (topk_masking).

v2: group-software-pipelined single pass.
 - Selector nets in 3-term fp16 split matmuls (hi/lo weights + dual-h /
   x-lo corrections) -> c = sigmoid*sigmoid in f32.
 - Per-row exact top-K=358: PE-transpose c to row-major (f32), per-row
   mean accumulated by the ACT eviction pass, affine-calibrated center
   t0, fp16 residuals, ITERS-step bisection with per-column counts
   split ~evenly between DVE (scalar_tensor_tensor+accum) and ACT
   (Sign+accum) - both run ~0.7us/[128,512] (accum forces 1x mode).
 - Mask = (resid >= lo), PE-transposed back, applied to xh, recon MLP,
   fp16 output (+br2 on device), un-transposed on host.
Phases are interleaved by group (A: selector+residuals, B: bisection,
C: mask+recon) so PE/ACT/DVE overlap.
"""

import sys

sys.path.insert(0, "/opt/trn_rl_repo")
import numpy as np

D = 512
H = 128
K = 358
B = 65536
NCORES = 8
R = B // NCORES
CHUNK = 512
NCHUNK = R // CHUNK      # 16
NCOL = R // 128          # 64
NG = 4
GC = NCOL // NG          # 16 cols per group
CPG = NCHUNK // NG       # 4 chunks per group
W_WIN = 0.0075
ITERS = 10
NA = 7                   # ACT-counted cols per group (rest DVE)
OSPLIT = [(0, 0, 128), (1, 128, 128), (2, 256, 102)]

_cache = {}


def _f16(a):
    return np.asarray(a, np.float16)


def _split16(a):
    hi = _f16(a)
    lo = _f16(np.asarray(a, np.float32) - hi.astype(np.float32))
    return hi, lo


def _sig(a):
    return 1.0 / (1.0 + np.exp(-a))


def _calibrate(x, P):
    """Simulate the device c-pipeline on 512 rows; fit thr ~ A*mu + C."""
    xs = np.asarray(x[:512], np.float32)
    xh = _f16(xs)
    xl = _f16(xs - xh.astype(np.float32))

    def mm3(ah, al, Wm):
        wh, wl = _split16(Wm)
        out = ah.astype(np.float32) @ wh.astype(np.float32)
        out = out + ah.astype(np.float32) @ wl.astype(np.float32)
        if al is not None:
            out = out + al.astype(np.float32) @ wh.astype(np.float32)
        return out

    def ev(a):
        h = np.maximum(a, 0)
        hh = _f16(h)
        return hh, _f16(h - hh.astype(np.float32))

    h1h, h1l = ev(mm3(xh, xl, P["W1"]) + P["b1"])
    h2h, h2l = ev(mm3(h1h, h1l, P["W2"]) + P["b2"])
    imp = _sig(mm3(h2h, h2l, P["W3"]) + P["b3"])
    g1h, g1l = ev(mm3(xh, xl, P["Wg1"]) + P["bg1"])
    gate = _sig(mm3(g1h, g1l, P["Wg2"]) + P["bg2"])
    c = (imp * gate).astype(np.float32)
    mu = c.mean(1)
    thr = np.partition(c, D - K, axis=1)[:, D - K]
    A1, C1 = np.polyfit(mu, thr, 1)
    return float(A1), float(C1)


def _build_program():
    from concourse import bacc, mybir, tile

    f32 = mybir.dt.float32
    fp16 = mybir.dt.float16
    Act = mybir.ActivationFunctionType
    Alu = mybir.AluOpType

    nc = bacc.Bacc("TRN2", target_bir_lowering=False, debug=False,
                   num_devices=NCORES)

    def din(name, shape, dt=fp16):
        return nc.dram_tensor(name, shape, dt, kind="ExternalInput").ap()

    xh_d = din("xh", [4, 128, R])
    xl_d = din("xl", [4, 128, R])
    w1h_d = din("w1h", [4, 128, 128]); w1l_d = din("w1l", [4, 128, 128])
    w2h_d = din("w2h", [128, 128]);    w2l_d = din("w2l", [128, 128])
    w3h_d = din("w3h", [4, 128, 128]); w3l_d = din("w3l", [4, 128, 128])
    wg1h_d = din("wg1h", [4, 128, 128]); wg1l_d = din("wg1l", [4, 128, 128])
    wg2h_d = din("wg2h", [4, 128, 128]); wg2l_d = din("wg2l", [4, 128, 128])
    wr1_d = din("wr1", [4, 128, 128])
    wr2_d = din("wr2", [128, K])
    b1_d = din("b1", [128, 1], f32)
    b2_d = din("b2", [128, 1], f32)
    b3_d = din("b3", [128, 4], f32)
    bg1_d = din("bg1", [128, 1], f32)
    bg2_d = din("bg2", [128, 4], f32)
    br1_d = din("br1", [128, 1], f32)
    br2_d = din("br2", [128, 3], f32)
    coef_d = din("coef", [128, 2], f32)
    idf32_d = din("idf32", [128, 128], f32)
    idf16_d = din("idf16", [128, 128], fp16)
    out_d = nc.dram_tensor("out", [3, 128, R], fp16, kind="ExternalOutput").ap()

    with tile.TileContext(nc) as tc:
        with (
            tc.tile_pool(name="wts", bufs=1) as wts,
            tc.tile_pool(name="big", bufs=1) as big,
            tc.tile_pool(name="xls", bufs=2) as xls,
            tc.tile_pool(name="hbuf", bufs=2) as hbuf,
            tc.tile_pool(name="sgbuf", bufs=1) as sgbuf,
            tc.tile_pool(name="crmb", bufs=2) as crmb,
            tc.tile_pool(name="mkb", bufs=2) as mkb,
            tc.tile_pool(name="rrb", bufs=2) as rrb,
            tc.tile_pool(name="obuf", bufs=2) as obuf,
            tc.tile_pool(name="st", bufs=1) as st,
            tc.tile_pool(name="ps_h", bufs=2, space="PSUM") as ps_h,
            tc.tile_pool(name="ps_s", bufs=2, space="PSUM") as ps_s,
            tc.tile_pool(name="ps_t", bufs=2, space="PSUM") as ps_t,
            tc.tile_pool(name="ps_m", bufs=2, space="PSUM") as ps_m,
        ):
            def ldt(dram, tiles, tag, dt=fp16):
                t = wts.tile([128, tiles, 128], dt, tag=tag)
                nc.sync.dma_start(t, dram.rearrange("t p m -> p t m"))
                return t

            def ld2(dram, shape, tag, dt=f32):
                t = wts.tile(shape, dt, tag=tag)
                nc.sync.dma_start(t, dram)
                return t

            w1h = ldt(w1h_d, 4, "w1h"); w1l = ldt(w1l_d, 4, "w1l")
            w2h = ld2(w2h_d, [128, 128], "w2h", fp16)
            w2l = ld2(w2l_d, [128, 128], "w2l", fp16)
            w3h = ldt(w3h_d, 4, "w3h"); w3l = ldt(w3l_d, 4, "w3l")
            wg1h = ldt(wg1h_d, 4, "wg1h"); wg1l = ldt(wg1l_d, 4, "wg1l")
            wg2h = ldt(wg2h_d, 4, "wg2h"); wg2l = ldt(wg2l_d, 4, "wg2l")
            wr1 = ldt(wr1_d, 4, "wr1")
            wr2 = ld2(wr2_d, [128, K], "wr2", fp16)
            b1 = ld2(b1_d, [128, 1], "b1"); b2 = ld2(b2_d, [128, 1], "b2")
            b3 = ld2(b3_d, [128, 4], "b3")
            bg1 = ld2(bg1_d, [128, 1], "bg1")
            bg2 = ld2(bg2_d, [128, 4], "bg2")
            br1 = ld2(br1_d, [128, 1], "br1")
            br2 = ld2(br2_d, [128, 3], "br2")
            coef = ld2(coef_d, [128, 2], "coef")
            idf32 = ld2(idf32_d, [128, 128], "idf32")
            idf16 = ld2(idf16_d, [128, 128], "idf16", fp16)

            xh = big.tile([128, 4, R], fp16, tag="xh")
            nc.sync.dma_start(xh, xh_d.rearrange("f p r -> p f r"))
            ebuf = big.tile([128, NCOL, 512], fp16, tag="ebuf")
            junkD = big.tile([128, 512], fp16, tag="junkD")
            ones16 = big.tile([128, 512], fp16, tag="ones16")
            nc.vector.memset(ones16, 1.0)
            junkA = big.tile([128, 512], fp16, tag="junkA")

            lo_g, hi_g, tmp_g, cnt_g, gek_g, gekn_g, ng_g = [], [], [], [], [], [], []
            t0n = st.tile([128, NCOL], f32, tag="t0n")
            mus = st.tile([128, NCOL], f32, tag="mus")
            u32 = mybir.dt.uint32
            for g in range(NG):
                for lst, nm, dt_ in (
                        (lo_g, "lo", f32), (hi_g, "hi", f32),
                        (tmp_g, "tmp", f32), (cnt_g, "cnt", f32),
                        (gek_g, "gek", u32), (gekn_g, "gekn", u32),
                        (ng_g, "ng", f32)):
                    lst.append(st.tile([128, GC], dt_, tag="%s%d" % (nm, g),
                                       name="%s%d" % (nm, g)))

            def evict_dual(psum, bias, dual):
                """relu(psum)+bias via ACT (hh) and DVE residual (hl)."""
                hh = hbuf.tile([128, CHUNK], fp16, tag="hh")
                nc.scalar.activation(hh, psum, Act.Relu, bias=bias)
                if not dual:
                    return hh, None
                hl = hbuf.tile([128, CHUNK], fp16, tag="hl")
                nc.vector.scalar_tensor_tensor(
                    hl, psum, 0.0, hh, op0=Alu.max, op1=Alu.subtract)
                return hh, hl

            def net3(stats, movs, psum):
                ops = []
                for (sh_, sl_), (mh, ml) in zip(stats, movs):
                    ops.append((sh_, mh))
                    if ml is not None:
                        ops.append((sh_, ml))
                    ops.append((sl_, mh))
                n = len(ops)
                for i, (sta, mov) in enumerate(ops):
                    nc.tensor.matmul(psum, lhsT=sta, rhs=mov,
                                     start=(i == 0), stop=(i == n - 1))

            # =============== phase A ===============
            def phase_a(ck):
                r0 = ck * CHUNK
                xhc = [xh[:, ft, r0:r0 + CHUNK] for ft in range(4)]
                xlt = xls.tile([128, 4, CHUNK], fp16, tag="xl")
                for ft in range(4):
                    nc.sync.dma_start(xlt[:, ft, :], xl_d[ft, :, r0:r0 + CHUNK])
                xmov = [(xhc[ft], xlt[:, ft, :]) for ft in range(4)]

                p = ps_h.tile([128, CHUNK], f32, tag="h")
                net3([(w1h[:, ki, :], w1l[:, ki, :]) for ki in range(4)],
                     xmov, p)
                h1h, h1l = evict_dual(p, b1, True)

                p = ps_h.tile([128, CHUNK], f32, tag="h")
                net3([(w2h, w2l)], [(h1h, h1l)], p)
                h2h, h2l = evict_dual(p, b2, True)

                sa = []
                for mt in range(4):
                    pw = ps_s.tile([128, CHUNK], f32, tag="s")
                    net3([(w3h[:, mt, :], w3l[:, mt, :])], [(h2h, h2l)], pw)
                    t = sgbuf.tile([128, CHUNK], f32, tag="sa%d" % mt,
                                   name="sa%d" % mt)
                    nc.scalar.activation(t, pw, Act.Sigmoid,
                                         bias=b3[:, mt:mt + 1])
                    sa.append(t)

                p = ps_h.tile([128, CHUNK], f32, tag="h")
                net3([(wg1h[:, ki, :], wg1l[:, ki, :]) for ki in range(4)],
                     xmov, p)
                g1h, g1l = evict_dual(p, bg1, True)

                ct = []
                for mt in range(4):
                    pw = ps_s.tile([128, CHUNK], f32, tag="s")
                    net3([(wg2h[:, mt, :], wg2l[:, mt, :])], [(g1h, g1l)], pw)
                    t = sgbuf.tile([128, CHUNK], f32, tag="sg%d" % (mt % 2),
                                   name="sg%d" % (mt % 2))
                    nc.scalar.activation(t, pw, Act.Sigmoid,
                                         bias=bg2[:, mt:mt + 1])
                    c = sgbuf.tile([128, CHUNK], f32, tag="c%d" % mt,
                                   name="c%d" % mt)
                    nc.vector.tensor_mul(c, sa[mt], t)
                    ct.append(c)

                for rt in range(4):
                    col = ck * 4 + rt
                    ptr = ps_t.tile([128, CHUNK], f32, tag="tr")
                    for mt in range(4):
                        nc.tensor.transpose(
                            ptr[:, mt * 128:(mt + 1) * 128],
                            ct[mt][:, rt * 128:(rt + 1) * 128], idf32)
                    crm = crmb.tile([128, CHUNK], f32, tag="crm")
                    nc.scalar.activation(crm, ptr, Act.Identity,
                                         accum_out=mus[:, col:col + 1])
                    nc.vector.tensor_scalar(
                        t0n[:, col:col + 1], mus[:, col:col + 1],
                        coef[:, 0:1], coef[:, 1:2],
                        op0=Alu.mult, op1=Alu.add)
                    nc.vector.tensor_scalar(
                        ebuf[:, col, :], crm, t0n[:, col:col + 1], None,
                        op0=Alu.add)

            # =============== phase B ===============
            def phase_b_init(g):
                nc.vector.memset(lo_g[g], -W_WIN)
                nc.vector.memset(hi_g[g], W_WIN)

            def phase_b_iter(g, it):
                lo, hi, tmp = lo_g[g], hi_g[g], tmp_g[g]
                cnt, gek, gekn, ngt = cnt_g[g], gek_g[g], gekn_g[g], ng_g[g]
                nc.vector.tensor_add(tmp, lo, hi)
                nc.vector.tensor_scalar_mul(tmp, tmp, 0.5)   # tmp = mid
                if NA:
                    nc.vector.tensor_scalar_mul(
                        ngt[:, GC - NA:], tmp[:, GC - NA:], -1.0)
                for i in range(GC - NA):
                    col = g * GC + i
                    nc.vector.scalar_tensor_tensor(
                        junkD, ebuf[:, col, :], tmp[:, i:i + 1], ones16,
                        op0=Alu.is_ge, op1=Alu.mult,
                        accum_out=cnt[:, i:i + 1])
                for i in range(GC - NA, GC):
                    col = g * GC + i
                    nc.scalar.activation(
                        junkA, ebuf[:, col, :], Act.Sign,
                        bias=ngt[:, i:i + 1],
                        accum_out=cnt[:, i:i + 1])
                nc.vector.tensor_scalar(
                    gek[:, :GC - NA], cnt[:, :GC - NA], float(K), None,
                    op0=Alu.is_ge)
                nc.vector.tensor_scalar(
                    gekn[:, :GC - NA], cnt[:, :GC - NA], float(K), None,
                    op0=Alu.is_lt)
                if NA:
                    thv = float(2 * K - 512)
                    nc.vector.tensor_scalar(
                        gek[:, GC - NA:], cnt[:, GC - NA:], thv, None,
                        op0=Alu.is_ge)
                    nc.vector.tensor_scalar(
                        gekn[:, GC - NA:], cnt[:, GC - NA:], thv, None,
                        op0=Alu.is_lt)
                nc.vector.copy_predicated(lo, gek, tmp)
                nc.vector.copy_predicated(hi, gekn, tmp)

            # =============== phase C ===============
            def phase_c(ck):
                g = ck // CPG
                r0 = ck * CHUNK
                mk = mkb.tile([128, 4, 512], fp16, tag="mk")
                for rt in range(4):
                    col = ck * 4 + rt
                    nc.vector.tensor_scalar(
                        mk[:, rt, :], ebuf[:, col, :],
                        lo_g[g][:, col - g * GC:col - g * GC + 1], None,
                        op0=Alu.is_ge)
                masked = []
                for ft in range(4):
                    pm = ps_m.tile([128, CHUNK], fp16, tag="pm")
                    for rt in range(4):
                        nc.tensor.transpose(
                            pm[:, rt * 128:(rt + 1) * 128],
                            mk[:, rt, ft * 128:(ft + 1) * 128], idf16)
                    mkd = mkb.tile([128, CHUNK], fp16, tag="mkd")
                    nc.vector.tensor_mul(mkd, pm, xh[:, ft, r0:r0 + CHUNK])
                    masked.append(mkd)

                p = ps_h.tile([128, CHUNK], f32, tag="h")
                for ft in range(4):
                    nc.tensor.matmul(p, lhsT=wr1[:, ft, :], rhs=masked[ft],
                                     start=(ft == 0), stop=(ft == 3))
                rr = rrb.tile([128, CHUNK], fp16, tag="rr")
                nc.scalar.activation(rr, p, Act.Relu, bias=br1)

                for (ot, o0, ow) in OSPLIT:
                    po = ps_h.tile([128, CHUNK], f32, tag="h")
                    nc.tensor.matmul(po[0:ow, :], lhsT=wr2[:, o0:o0 + ow],
                                     rhs=rr, start=True, stop=True)
                    of = obuf.tile([128, CHUNK], fp16, tag="of")
                    nc.scalar.activation(
                        of[0:ow, :], po[0:ow, :], Act.Identity,
                        bias=br2[0:ow, ot:ot + 1])
                    nc.sync.dma_start(out_d[ot, 0:ow, r0:r0 + CHUNK],
                                      of[0:ow, :])

            # =============== emission (software pipeline) ===============
            IT_SL = [(ITERS * s // CPG, ITERS * (s + 1) // CPG)
                     for s in range(CPG)]
            for g in range(NG):
                phase_b_init(g)
            for ck in range(NCHUNK):
                g, sl = ck // CPG, ck % CPG
                phase_a(ck)
                if g >= 1:
                    for it in range(*IT_SL[sl]):
                        phase_b_iter(g - 1, it)
                if g >= 2:
                    phase_c(CPG * (g - 2) + sl)
            for sl in range(CPG):
                for it in range(*IT_SL[sl]):
                    phase_b_iter(NG - 1, it)
                phase_c(CPG * (NG - 2) + sl)
            for sl in range(CPG):
                phase_c(CPG * (NG - 1) + sl)

    nc.compile()
    return nc


def kernel(**inputs):
    from concourse.bass_utils import run_bass_kernel_spmd

    x = np.asarray(inputs["x"], np.float32)
    names = ["W1", "b1", "W2", "b2", "W3", "b3", "Wg1", "bg1", "Wg2", "bg2",
             "Wr1", "br1", "Wr2", "br2"]
    P = {n: np.asarray(inputs[n], np.float32) for n in names}

    A1, C1 = _calibrate(x, P)

    def ksplit(Wm):
        h, l = _split16(Wm)
        return (np.ascontiguousarray(h.reshape(4, 128, 128)),
                np.ascontiguousarray(l.reshape(4, 128, 128)))

    def msplit(Wm):
        h, l = _split16(Wm)
        return (np.ascontiguousarray(h.reshape(128, 4, 128).transpose(1, 0, 2)),
                np.ascontiguousarray(l.reshape(128, 4, 128).transpose(1, 0, 2)))

    w1h, w1l = ksplit(P["W1"])
    w2h, w2l = _split16(P["W2"])
    w3h, w3l = msplit(P["W3"])
    wg1h, wg1l = ksplit(P["Wg1"])
    wg2h, wg2l = msplit(P["Wg2"])
    wr1 = np.ascontiguousarray(_f16(P["Wr1"]).reshape(4, 128, 128))
    wr2 = np.ascontiguousarray(_f16(P["Wr2"]))
    coef = np.zeros((128, 2), np.float32)
    coef[:, 0] = -A1 / 512.0
    coef[:, 1] = -C1
    b3p = np.zeros((128, 4), np.float32)
    b3p[:] = P["b3"].reshape(4, 128).T
    bg2p = np.zeros((128, 4), np.float32)
    bg2p[:] = P["bg2"].reshape(4, 128).T
    br2t = np.zeros(384, np.float32)
    br2t[:K] = P["br2"]
    br2p = np.ascontiguousarray(br2t.reshape(3, 128).T)
    ident = np.eye(128)
    shared = dict(
        w1h=w1h, w1l=w1l,
        w2h=np.ascontiguousarray(w2h), w2l=np.ascontiguousarray(w2l),
        w3h=w3h, w3l=w3l,
        wg1h=wg1h, wg1l=wg1l, wg2h=wg2h, wg2l=wg2l,
        wr1=wr1, wr2=wr2,
        b1=P["b1"].reshape(128, 1), b2=P["b2"].reshape(128, 1),
        b3=b3p, bg1=P["bg1"].reshape(128, 1), bg2=bg2p,
        br1=P["br1"].reshape(128, 1), br2=br2p,
        coef=coef,
        idf32=ident.astype(np.float32),
        idf16=ident.astype(np.float16),
    )

    in_maps = []
    for i in range(NCORES):
        xs = x[i * R:(i + 1) * R]
        xT = np.ascontiguousarray(xs.T)
        xTh = _f16(xT)
        xTl = _f16(xT - xTh.astype(np.float32))
        m = dict(shared)
        m["xh"] = np.ascontiguousarray(xTh.reshape(4, 128, R))
        m["xl"] = np.ascontiguousarray(xTl.reshape(4, 128, R))
        in_maps.append(m)

    if "nc" not in _cache:
        _cache["nc"] = _build_program()
    nc = _cache["nc"]
    _cache["in_maps"] = in_maps

    res = run_bass_kernel_spmd(nc, in_maps, list(range(NCORES)))
    outs = []
    for i in range(NCORES):
        o = res.results[i]["out"].astype(np.float32)   # [3,128,R]
        o = o.reshape(384, R)[:K]
        outs.append(np.ascontiguousarray(o.T))
    return np.concatenate(outs, axis=0)


if __name__ == "__main__":
    rng = np.random.default_rng(0)
    fake = {"x": rng.standard_normal((B, D), dtype=np.float32)}
    s = lambda f: 1.0 / np.sqrt(f)
    for nm, sh, fan in [("W1", (D, H), D), ("W2", (H, H), H), ("W3", (H, D), H),
                        ("Wg1", (D, H), D), ("Wg2", (H, D), H),
                        ("Wr1", (D, H), D), ("Wr2", (H, K), H)]:
        fake[nm] = rng.uniform(-s(fan), s(fan), sh).astype(np.float32)
    for nm, sh in [("b1", H), ("b2", H), ("b3", D), ("bg1", H), ("bg2", D),
                   ("br1", H), ("br2", K)]:
        fake[nm] = np.zeros(sh, np.float32)
    out = kernel(**fake)
    print("out", out.shape, out.dtype, float(np.abs(out).max()))



# revision 2
# speedup vs baseline: 1.0156x; 1.0156x over previous
"""Trainium2 Bass kernel for AdaptiveFeatureSelector (topk_masking).

v3: row-major scores + secant threshold search + max8 exact finish.
 - Selector nets in 3-term fp16 split matmuls. Score layers (W3/Wg2)
   computed ROW-major (stationary = h2/g1 row-slices, moving = full W3)
   so c lands [rows, feat] in psum: no f32 PE transposes, no crm pass.
 - mu accumulated by the DVE product op (imp*gate, accum_out); t0
   affine-calibrated; resid (c - t0) stored fp16 in ebuf.
 - Threshold search: 4 count passes (DVE is_ge+accum / ACT Sign+accum)
   with secant stepping toward count K-4; rows whose probe hits
   cnt in [K-8, K-1] are "armed". Finish: mask elements >= armed probe
   to -1e4, DVE max8 -> top-8 below it, one-hot extract the
   (K-1-cnt)-th -> EXACT per-row threshold t*. Fallback rows use the
   tightest >=K bracket bound.
 - Phase C: mask = (resid >= t*), PE-transpose fp16, recon MLP, fp16 out.
"""

import sys

sys.path.insert(0, "/opt/trn_rl_repo")
import numpy as np

D = 512
H = 128
K = 358
B = 65536
NCORES = 8
R = B // NCORES
CHUNK = 512
NCHUNK = R // CHUNK      # 16
NCOL = R // 128          # 64
NG = 4
GC = NCOL // NG          # 16 cols per group
CPG = NCHUNK // NG       # 4 chunks per group
W_WIN = 0.008
NPASS = 4
NA = 8                   # ACT-counted cols per group (rest DVE)
KTGT = float(K - 4)      # probe target count
OSPLIT = [(0, 0, 128), (1, 128, 128), (2, 256, 102)]

_cache = {}


def _f16(a):
    return np.asarray(a, np.float16)


def _split16(a):
    hi = _f16(a)
    lo = _f16(np.asarray(a, np.float32) - hi.astype(np.float32))
    return hi, lo


def _sig(a):
    return 1.0 / (1.0 + np.exp(-a))


def _calibrate(x, P):
    """Simulate the device c-pipeline on 512 rows; fit thr ~ A*mu + C,
    and estimate the local order-stat density near the threshold."""
    xs = np.asarray(x[:512], np.float32)
    xh = _f16(xs)
    xl = _f16(xs - xh.astype(np.float32))

    def mm3(ah, al, Wm):
        wh, wl = _split16(Wm)
        out = ah.astype(np.float32) @ wh.astype(np.float32)
        out = out + ah.astype(np.float32) @ wl.astype(np.float32)
        if al is not None:
            out = out + al.astype(np.float32) @ wh.astype(np.float32)
        return out

    def ev(a):
        h = np.maximum(a, 0)
        hh = _f16(h)
        return hh, _f16(h - hh.astype(np.float32))

    h1h, h1l = ev(mm3(xh, xl, P["W1"]) + P["b1"])
    h2h, h2l = ev(mm3(h1h, h1l, P["W2"]) + P["b2"])
    imp = _sig(mm3(h2h, h2l, P["W3"]) + P["b3"])
    g1h, g1l = ev(mm3(xh, xl, P["Wg1"]) + P["bg1"])
    gate = _sig(mm3(g1h, g1l, P["Wg2"]) + P["bg2"])
    c = (imp * gate).astype(np.float32)
    mu = c.mean(1)
    thr = np.partition(c, D - K, axis=1)[:, D - K]
    A1, C1 = np.polyfit(mu, thr, 1)
    cs = np.sort(c, axis=1)
    dbar = np.median(8.0 / (cs[:, D - K + 4] - cs[:, D - K - 4]))
    return float(A1), float(C1), float(dbar)


def _build_program(has_b3, has_bg2):
    from concourse import bacc, mybir, tile

    f32 = mybir.dt.float32
    fp16 = mybir.dt.float16
    Act = mybir.ActivationFunctionType
    Alu = mybir.AluOpType

    nc = bacc.Bacc("TRN2", target_bir_lowering=False, debug=False,
                   num_devices=NCORES)

    def din(name, shape, dt=fp16):
        return nc.dram_tensor(name, shape, dt, kind="ExternalInput").ap()

    # xh quarters / xl chunks laid out partition-major for contiguous
    # per-partition DMA runs (few descriptors)
    xh_d = din("xh", [4, 128, 4, R // 4])
    xl_d = din("xl", [NCHUNK, 128, 4, CHUNK])
    w1h_d = din("w1h", [4, 128, 128]); w1l_d = din("w1l", [4, 128, 128])
    w2h_d = din("w2h", [128, 128]);    w2l_d = din("w2l", [128, 128])
    w3h_d = din("w3h", [128, 512]);    w3l_d = din("w3l", [128, 512])
    wg2h_d = din("wg2h", [128, 512]);  wg2l_d = din("wg2l", [128, 512])
    wg1h_d = din("wg1h", [4, 128, 128]); wg1l_d = din("wg1l", [4, 128, 128])
    wr1_d = din("wr1", [4, 128, 128])
    wr2_d = din("wr2", [128, K])
    b1_d = din("b1", [128, 1], f32)
    b2_d = din("b2", [128, 1], f32)
    bg1_d = din("bg1", [128, 1], f32)
    br1_d = din("br1", [128, 1], f32)
    br2_d = din("br2", [128, 3], f32)
    b3r_d = din("b3r", [1, 512]) if has_b3 else None
    bg2r_d = din("bg2r", [1, 512]) if has_bg2 else None
    coef_d = din("coef", [128, 8], f32)   # [0]=-A/512 [1]=-C [2]=1/dbar
    iota8_d = din("iota8", [128, GC * 8])
    idf16_d = din("idf16", [128, 128], fp16)
    out_d = nc.dram_tensor("out", [3, 128, R], fp16,
                           kind="ExternalOutput").ap()

    with tile.TileContext(nc) as tc:
        with (
            tc.tile_pool(name="wts", bufs=1) as wts,
            tc.tile_pool(name="big", bufs=1) as big,
            tc.tile_pool(name="xls", bufs=2) as xls,
            tc.tile_pool(name="hbuf", bufs=2) as hbuf,
            tc.tile_pool(name="sgb", bufs=3) as sgb,
            tc.tile_pool(name="mkb", bufs=2) as mkb,
            tc.tile_pool(name="rrb", bufs=2) as rrb,
            tc.tile_pool(name="obuf", bufs=2) as obuf,
            tc.tile_pool(name="fin", bufs=2) as fin,
            tc.tile_pool(name="st", bufs=1) as st,
            tc.tile_pool(name="ps_h", bufs=2, space="PSUM") as ps_h,
            tc.tile_pool(name="ps_s", bufs=2, space="PSUM") as ps_s,
            tc.tile_pool(name="ps_m", bufs=1, space="PSUM") as ps_m,
        ):
            def ldt(dram, tiles, tag, dt=fp16):
                t = wts.tile([128, tiles, 128], dt, tag=tag, name=tag)
                nc.sync.dma_start(t, dram.rearrange("t p m -> p t m"))
                return t

            def ld2(dram, shape, tag, dt=f32):
                t = wts.tile(shape, dt, tag=tag, name=tag)
                nc.sync.dma_start(t, dram)
                return t

            w1h = ldt(w1h_d, 4, "w1h"); w1l = ldt(w1l_d, 4, "w1l")
            w2h = ld2(w2h_d, [128, 128], "w2h", fp16)
            w2l = ld2(w2l_d, [128, 128], "w2l", fp16)
            w3h = ld2(w3h_d, [128, 512], "w3h", fp16)
            w3l = ld2(w3l_d, [128, 512], "w3l", fp16)
            wg2h = ld2(wg2h_d, [128, 512], "wg2h", fp16)
            wg2l = ld2(wg2l_d, [128, 512], "wg2l", fp16)
            wg1h = ldt(wg1h_d, 4, "wg1h"); wg1l = ldt(wg1l_d, 4, "wg1l")
            wr1 = ldt(wr1_d, 4, "wr1")
            wr2 = ld2(wr2_d, [128, K], "wr2", fp16)
            b1 = ld2(b1_d, [128, 1], "b1"); b2 = ld2(b2_d, [128, 1], "b2")
            bg1 = ld2(bg1_d, [128, 1], "bg1")
            br1 = ld2(br1_d, [128, 1], "br1")
            br2 = ld2(br2_d, [128, 3], "br2")
            coef = ld2(coef_d, [128, 8], "coef")
            iota8 = ld2(iota8_d, [128, GC * 8], "iota8", fp16)
            idf16 = ld2(idf16_d, [128, 128], "idf16", fp16)
            b3r = ld2(b3r_d, [1, 512], "b3r", fp16) if has_b3 else None
            bg2r = ld2(bg2r_d, [1, 512], "bg2r", fp16) if has_bg2 else None
            ones1 = None
            if has_b3 or has_bg2:
                ones1 = wts.tile([1, 128], fp16, tag="ones1", name="ones1")
                nc.vector.memset(ones1, 1.0)

            # [128, quarter, f, r-in-quarter]: quarter slice contiguous per
            # partition on both sides -> 128-descriptor DMAs. Quarter q
            # loaded lazily (staggered) to not block chunk-0 xl.
            xh = big.tile([128, 4, 4, R // 4], fp16, tag="xh", name="xh")

            def load_xh(q):
                nc.sync.dma_start(xh[:, q, :, :], xh_d[q])

            def xhv(ck, ft):
                q, o = ck // 4, (ck % 4) * CHUNK
                return xh[:, q, ft, o:o + CHUNK]
            ebuf = big.tile([128, NCOL, 512], fp16, tag="ebuf", name="ebuf")
            junkD = big.tile([128, 512], fp16, tag="junkD", name="junkD")
            junkA = big.tile([128, 512], fp16, tag="junkA", name="junkA")

            # per-group state tiles [128, GC]
            def stt(nm, g, dt=f32, w=GC):
                return st.tile([128, w], dt, tag="%s%d" % (nm, g),
                               name="%s%d" % (nm, g))

            t_g = [stt("t", g) for g in range(NG)]
            tp_g = [stt("tp", g) for g in range(NG)]
            cp_g = [stt("cp", g) for g in range(NG)]
            cnt_g = [stt("cnt", g) for g in range(NG)]
            ngt_g = [stt("ngt", g) for g in range(NG)]
            lo_g = [stt("lo", g) for g in range(NG)]
            hiA_g = [stt("hiA", g) for g in range(NG)]
            chiA_g = [stt("chiA", g) for g in range(NG)]
            arm_g = [stt("arm", g) for g in range(NG)]
            ts_g = [stt("ts", g) for g in range(NG)]
            m8_g = [st.tile([128, GC, 8], fp16, tag="m8%d" % g,
                            name="m8%d" % g) for g in range(NG)]
            sc1_g = [stt("sc1", g) for g in range(NG)]
            sc2_g = [stt("sc2", g) for g in range(NG)]
            sc3_g = [stt("sc3", g) for g in range(NG)]
            u32 = mybir.dt.uint32
            pu1_g = [stt("pu1", g, u32) for g in range(NG)]
            pu2_g = [stt("pu2", g, u32) for g in range(NG)]
            mus = st.tile([128, NCOL], f32, tag="mus", name="mus")
            t0n = st.tile([128, NCOL], f32, tag="t0n", name="t0n")

            def evict_dual(psum, bias, tag, dual=True):
                hh = hbuf.tile([128, CHUNK], fp16, tag=tag + "h",
                               name=tag + "h")
                nc.scalar.activation(hh, psum, Act.Relu, bias=bias)
                if not dual:
                    return hh, None
                hl = hbuf.tile([128, CHUNK], fp16, tag=tag + "l",
                               name=tag + "l")
                nc.vector.scalar_tensor_tensor(
                    hl, psum, 0.0, hh, op0=Alu.max, op1=Alu.subtract)
                return hh, hl

            def net3(stats, movs, psum, start=True):
                ops = []
                for (sh_, sl_), (mh, ml) in zip(stats, movs):
                    ops.append((sh_, mh))
                    if ml is not None:
                        ops.append((sh_, ml))
                    if sl_ is not None:
                        ops.append((sl_, mh))
                n = len(ops)
                for i, (sta, mov) in enumerate(ops):
                    nc.tensor.matmul(psum, lhsT=sta, rhs=mov,
                                     start=(i == 0 and start),
                                     stop=(i == n - 1))

            # =============== phase A ===============
            def phase_a(ck, dfill, afill, pefill=None):
                """dfill/afill: callables(n) emitting up to n queued DVE/ACT
                thunks from the interleaved phase-B work list. pefill emits
                independent PE work (mask transposes) into the h2-eviction
                latency window to keep PE ramped."""
                xlt = xls.tile([128, 4, CHUNK], fp16, tag="xl", name="xl")
                nc.sync.dma_start(xlt, xl_d[ck])
                xmov = [(xhv(ck, ft), xlt[:, ft, :]) for ft in range(4)]

                pw1 = ps_h.tile([128, CHUNK], f32, tag="h", name="ph")
                net3([(w1h[:, ki, :], w1l[:, ki, :]) for ki in range(4)],
                     xmov, pw1)
                pg1 = ps_h.tile([128, CHUNK], f32, tag="h", name="ph")
                net3([(wg1h[:, ki, :], wg1l[:, ki, :]) for ki in range(4)],
                     xmov, pg1)
                afill(2)
                dfill(4)
                h1h, _ = evict_dual(pw1, b1, "h1", dual=False)
                g1h, _ = evict_dual(pg1, bg1, "g1", dual=False)

                p = ps_h.tile([128, CHUNK], f32, tag="h", name="ph")
                net3([(w2h, None)], [(h1h, None)], p)
                if pefill is not None:
                    pefill()
                afill(2)
                dfill(4)
                h2h, _ = evict_dual(p, b2, "h2", dual=False)

                resid_q = []

                def emit_resid():
                    col_, ct_ = resid_q.pop(0)
                    nc.scalar.activation(
                        ebuf[:, col_, :], ct_, Act.Identity,
                        bias=t0n[:, col_:col_ + 1])

                for rt in range(4):
                    col = ck * 4 + rt
                    rs = rt * 128
                    # imp+gate scores row-major into one 2-bank psum pair
                    ps2 = ps_s.tile([128, 2, CHUNK], f32, tag="s", name="ps2")
                    pi, pg = ps2[:, 0, :], ps2[:, 1, :]
                    if has_b3:
                        nc.tensor.matmul(pi, lhsT=ones1, rhs=b3r,
                                         start=True, stop=False)
                    net3([(h2h[:, rs:rs + 128], None)],
                         [(w3h, None)], pi, start=not has_b3)
                    if has_bg2:
                        nc.tensor.matmul(pg, lhsT=ones1, rhs=bg2r,
                                         start=True, stop=False)
                    net3([(g1h[:, rs:rs + 128], None)],
                         [(wg2h, wg2l)], pg, start=not has_bg2)
                    # one fused sigmoid eviction for both
                    sg2 = sgb.tile([128, 2, CHUNK], f32, tag="sg2",
                                   name="sg2")
                    nc.scalar.activation(sg2, ps2, Act.Sigmoid)
                    if resid_q:
                        emit_resid()   # one-rt-delayed: ct ready by now
                    afill(1)
                    dfill(2)
                    # c = imp*gate with accum -> mu
                    ct = sgb.tile([128, CHUNK], f32, tag="c", name="ct")
                    nc.vector.scalar_tensor_tensor(
                        ct, sg2[:, 0, :], 1.0, sg2[:, 1, :],
                        op0=Alu.mult, op1=Alu.mult,
                        accum_out=mus[:, col:col + 1])
                    # t0n = -(A*mu + C)  (coef0=-A/512, coef1=-C)
                    nc.vector.tensor_scalar(
                        t0n[:, col:col + 1], mus[:, col:col + 1],
                        coef[:, 0:1], coef[:, 1:2],
                        op0=Alu.mult, op1=Alu.add)
                    resid_q.append((col, ct))
                emit_resid()

            # =============== phase B ===============
            def phase_b_init(g):
                nc.vector.memset(t_g[g], 0.0)
                nc.vector.memset(ngt_g[g], 0.0)
                nc.vector.memset(lo_g[g], -W_WIN)
                nc.vector.memset(arm_g[g], 0.0)
                nc.vector.memset(hiA_g[g], W_WIN)
                nc.vector.memset(chiA_g[g], float(K - 8))

            def phase_b_count_thunks(g, p):
                t, cnt, ngt = t_g[g], cnt_g[g], ngt_g[g]
                dth, ath = [], []
                for i in range(NA):
                    col = g * GC + i
                    def a_th(i=i, col=col):
                        nc.scalar.activation(
                            junkA, ebuf[:, col, :], Act.Sign,
                            bias=ngt[:, i:i + 1],
                            accum_out=cnt[:, i:i + 1])
                    ath.append(a_th)
                for i in range(NA, GC):
                    col = g * GC + i
                    def d_th(i=i, col=col):
                        nc.vector.tensor_scalar(
                            junkD, ebuf[:, col, :], t[:, i:i + 1], None,
                            op0=Alu.is_ge, op1=Alu.add,
                            accum_out=cnt[:, i:i + 1])
                    dth.append(d_th)
                return dth, ath

            def phase_b_smalls(g, p):
                t, tp, cp, cnt = t_g[g], tp_g[g], cp_g[g], cnt_g[g]
                lo, hiA, chiA, arm = lo_g[g], hiA_g[g], chiA_g[g], arm_g[g]
                ngt, s1, s2, s3 = ngt_g[g], sc1_g[g], sc2_g[g], sc3_g[g]
                if NA:
                    # ACT counted sign-sums: cnt = 0.5*s + 256
                    nc.vector.tensor_scalar(
                        cnt[:, :NA], cnt[:, :NA], 0.5, 256.0,
                        op0=Alu.mult, op1=Alu.add)
                pu1, pu2 = pu1_g[g], pu2_g[g]
                # bracket lo = max over probes with cnt >= K
                nc.vector.tensor_scalar(pu1, cnt, float(K), None,
                                        op0=Alu.is_ge)
                nc.vector.tensor_tensor(s3, lo, t, op=Alu.max)
                nc.vector.copy_predicated(lo, pu1, s3)
                # arming: s1 = (cnt >= K-8) * (cnt < K); new = s1 * (1-arm)
                nc.vector.tensor_scalar(s1, cnt, float(K - 8), None,
                                        op0=Alu.is_ge)
                nc.vector.tensor_scalar(s2, cnt, float(K), None,
                                        op0=Alu.is_lt)
                nc.vector.tensor_tensor(s1, s1, s2, op=Alu.mult)
                nc.vector.tensor_scalar(s2, arm, 0.5, None, op0=Alu.is_lt)
                nc.vector.tensor_tensor(s2, s1, s2, op=Alu.mult)
                nc.vector.tensor_scalar(pu2, s2, 0.5, None, op0=Alu.is_ge)
                nc.vector.copy_predicated(hiA, pu2, t)
                nc.vector.copy_predicated(chiA, pu2, cnt)
                nc.vector.tensor_tensor(arm, arm, s1, op=Alu.max)
                if p == NPASS - 1:
                    return
                # step
                if p == 0:
                    # t1 = clip(t + (cnt - KTGT)/dbar)
                    nc.vector.tensor_scalar(
                        s1, cnt, KTGT, coef[:, 2:3],
                        op0=Alu.subtract, op1=Alu.mult)
                else:
                    # secant: d = (cp - cnt)/(t - tp); step=(cnt-KTGT)/d
                    nc.vector.tensor_tensor(s1, cp, cnt, op=Alu.subtract)
                    nc.vector.tensor_scalar(s1, s1, 0.25, None, op0=Alu.add)
                    nc.vector.reciprocal(s2, s1)           # 1/(cp-cnt)
                    nc.vector.tensor_tensor(s3, t, tp, op=Alu.subtract)
                    nc.vector.tensor_tensor(s2, s2, s3, op=Alu.mult)
                    # s2 = (t-tp)/(cp-cnt) = 1/d ; clip to [1/(5 dbar), 1/(0.3 dbar)]
                    nc.vector.tensor_scalar(s2, s2, coef[:, 3:4], None,
                                            op0=Alu.max)
                    nc.vector.tensor_scalar(s2, s2, coef[:, 4:5], None,
                                            op0=Alu.min)
                    nc.vector.tensor_scalar(s1, cnt, KTGT, None,
                                            op0=Alu.subtract)
                    nc.vector.tensor_tensor(s1, s1, s2, op=Alu.mult)
                nc.vector.tensor_copy(tp, t)
                nc.vector.tensor_copy(cp, cnt)
                nc.vector.tensor_tensor(t, t, s1, op=Alu.add)
                nc.vector.tensor_scalar(t, t, W_WIN, -W_WIN,
                                        op0=Alu.min, op1=Alu.max)
                if NA:
                    nc.vector.tensor_scalar_mul(ngt[:, :NA], t[:, :NA], -1.0)

            def phase_b_fin_slice(g, sl4):
                """Finish cols [4*sl4, 4*sl4+4) of group g."""
                hiA, chiA, arm, lo = hiA_g[g], chiA_g[g], arm_g[g], lo_g[g]
                ts, s1 = ts_g[g], sc1_g[g]
                pu1 = pu1_g[g]
                m8 = m8_g[g]
                i0 = 4 * sl4
                for i in range(i0, i0 + 4):
                    col = g * GC + i
                    ex = fin.tile([128, 512], fp16, tag="ex", name="ex")
                    nc.vector.tensor_scalar(
                        ex, ebuf[:, col, :], hiA[:, i:i + 1], -1e4,
                        op0=Alu.is_ge, op1=Alu.mult)
                    nc.vector.tensor_tensor(ex, ex, ebuf[:, col, :],
                                            op=Alu.add)
                    nc.vector.max(m8[:, i, :], ex)
                # idx = (K-1) - chiA  -> one-hot -> t* = m8[idx]
                sl_ = slice(i0, i0 + 4)
                nc.vector.tensor_scalar(s1[:, sl_], chiA[:, sl_],
                                        float(K - 1), -1.0,
                                        op0=Alu.subtract, op1=Alu.mult)
                oh = fin.tile([128, 4, 8], fp16, tag="oh", name="oh")
                i8v = iota8.rearrange("p (a b) -> p a b", b=8)[:, sl_, :]
                s1b = s1[:, sl_].unsqueeze(-1).broadcast_to([128, 4, 8])
                nc.vector.tensor_tensor(oh, i8v, s1b, op=Alu.is_equal)
                nc.vector.tensor_tensor(oh, oh, m8[:, sl_, :], op=Alu.mult)
                nc.vector.tensor_reduce(ts[:, sl_], oh, mybir.AxisListType.X,
                                        Alu.add)
                # fallback: not armed -> lo
                nc.vector.tensor_scalar(pu1[:, sl_], arm[:, sl_], 0.5, None,
                                        op0=Alu.is_lt)
                nc.vector.copy_predicated(ts[:, sl_], pu1[:, sl_],
                                          lo[:, sl_])

            # =============== phase C ===============
            _mk_pending = {}

            def phase_c_mask(ck):
                g = ck // CPG
                ts = ts_g[g]
                mk = mkb.tile([128, 4, 512], fp16, tag="mk", name="mk")
                for rt in range(4):
                    col = ck * 4 + rt
                    i = col - g * GC
                    nc.vector.tensor_scalar(
                        mk[:, rt, :], ebuf[:, col, :],
                        ts[:, i:i + 1], None, op0=Alu.is_ge)
                _mk_pending[ck] = mk

            _pm_pending = {}

            def phase_c_trans(ck):
                mk = _mk_pending.pop(ck)
                pm = ps_m.tile([128, 4, CHUNK], fp16, tag="pm", name="pm")
                for ft in range(4):
                    for rt in range(4):
                        nc.tensor.transpose(
                            pm[:, ft, rt * 128:(rt + 1) * 128],
                            mk[:, rt, ft * 128:(ft + 1) * 128], idf16)
                _pm_pending[ck] = pm

            def phase_c(ck):
                r0 = ck * CHUNK
                pm = _pm_pending.pop(ck)
                masked = []
                for ft in range(4):
                    mkd = mkb.tile([128, CHUNK], fp16, tag="mkd", name="mkd")
                    nc.vector.tensor_mul(mkd, pm[:, ft, :], xhv(ck, ft))
                    masked.append(mkd)

                p = ps_h.tile([128, CHUNK], f32, tag="h", name="ph")
                for ft in range(4):
                    nc.tensor.matmul(p, lhsT=wr1[:, ft, :], rhs=masked[ft],
                                     start=(ft == 0), stop=(ft == 3))
                rr = rrb.tile([128, CHUNK], fp16, tag="rr", name="rr")
                nc.scalar.activation(rr, p, Act.Relu, bias=br1)

                for (ot, o0, ow) in OSPLIT:
                    po = ps_h.tile([128, CHUNK], f32, tag="h", name="ph")
                    nc.tensor.matmul(po[0:ow, :], lhsT=wr2[:, o0:o0 + ow],
                                     rhs=rr, start=True, stop=True)
                    of = obuf.tile([128, CHUNK], fp16, tag="of", name="of")
                    nc.scalar.activation(
                        of[0:ow, :], po[0:ow, :], Act.Identity,
                        bias=br2[0:ow, ot:ot + 1])
                    nc.sync.dma_start(out_d[ot, 0:ow, r0:r0 + CHUNK],
                                      of[0:ow, :])

            # =============== emission (software pipeline) ===============
            # slot s of group g runs pass s of group g-1 (counts interleaved
            # into phase_a's matmul shadows); slot 3 also runs fin.
            class Filler:
                def __init__(self, th):
                    self.th = list(th)
                    self.i = 0

                def __call__(self, n):
                    while n > 0 and self.i < len(self.th):
                        self.th[self.i]()
                        self.i += 1
                        n -= 1

                def drain(self):
                    self(1 << 30)

            for g in range(NG):
                phase_b_init(g)
            load_xh(0)
            for ck in range(NCHUNK):
                g, sl = ck // CPG, ck % CPG
                if ck in (1, 5, 9):
                    load_xh(ck // 4 + 1)
                if g >= 2:
                    phase_b_fin_slice(g - 2, sl)
                    phase_c_mask(CPG * (g - 2) + sl)
                if g >= 1:
                    dth, ath = phase_b_count_thunks(g - 1, sl)
                else:
                    dth, ath = [], []
                df, af = Filler(dth), Filler(ath)
                ckc = CPG * (g - 2) + sl
                pef = (lambda: phase_c_trans(ckc)) if g >= 2 else None
                phase_a(ck, df, af, pef)
                df.drain()
                af.drain()
                if g >= 1:
                    phase_b_smalls(g - 1, sl)
                if g >= 2:
                    phase_c(ckc)
            for sl in range(CPG):
                phase_b_fin_slice(NG - 2, sl)
                phase_c_mask(CPG * (NG - 2) + sl)
                dth, ath = phase_b_count_thunks(NG - 1, sl)
                for th in ath + dth:
                    th()
                phase_b_smalls(NG - 1, sl)
                phase_c_trans(CPG * (NG - 2) + sl)
                phase_c(CPG * (NG - 2) + sl)
            for sl in range(CPG):
                phase_b_fin_slice(NG - 1, sl)
                phase_c_mask(CPG * (NG - 1) + sl)
                phase_c_trans(CPG * (NG - 1) + sl)
                phase_c(CPG * (NG - 1) + sl)

    nc.compile()
    return nc


def kernel(**inputs):
    from concourse.bass_utils import run_bass_kernel_spmd

    x = np.asarray(inputs["x"], np.float32)
    names = ["W1", "b1", "W2", "b2", "W3", "b3", "Wg1", "bg1", "Wg2", "bg2",
             "Wr1", "br1", "Wr2", "br2"]
    P = {n: np.asarray(inputs[n], np.float32) for n in names}

    A1, C1, dbar = _calibrate(x, P)
    has_b3 = bool(np.any(P["b3"] != 0))
    has_bg2 = bool(np.any(P["bg2"] != 0))

    def ksplit(Wm):
        h, l = _split16(Wm)
        return (np.ascontiguousarray(h.reshape(4, 128, 128)),
                np.ascontiguousarray(l.reshape(4, 128, 128)))

    w1h, w1l = ksplit(P["W1"])
    w2h, w2l = _split16(P["W2"])
    w3h, w3l = _split16(P["W3"])      # [128, 512] as-is
    wg1h, wg1l = ksplit(P["Wg1"])
    wg2h, wg2l = _split16(P["Wg2"])
    wr1 = np.ascontiguousarray(_f16(P["Wr1"]).reshape(4, 128, 128))
    wr2 = np.ascontiguousarray(_f16(P["Wr2"]))
    coef = np.zeros((128, 8), np.float32)
    coef[:, 0] = -A1 / 512.0
    coef[:, 1] = -C1
    coef[:, 2] = 1.0 / dbar
    coef[:, 3] = 1.0 / (5.0 * dbar)
    coef[:, 4] = 1.0 / (0.3 * dbar)
    br2t = np.zeros(384, np.float32)
    br2t[:K] = P["br2"]
    br2p = np.ascontiguousarray(br2t.reshape(3, 128).T)
    ident = np.eye(128)
    iota8 = np.tile(np.arange(8, dtype=np.float16), (128, GC))
    shared = dict(
        w1h=w1h, w1l=w1l,
        w2h=np.ascontiguousarray(w2h), w2l=np.ascontiguousarray(w2l),
        w3h=np.ascontiguousarray(w3h), w3l=np.ascontiguousarray(w3l),
        wg2h=np.ascontiguousarray(wg2h), wg2l=np.ascontiguousarray(wg2l),
        wg1h=wg1h, wg1l=wg1l,
        wr1=wr1, wr2=wr2,
        b1=P["b1"].reshape(128, 1), b2=P["b2"].reshape(128, 1),
        bg1=P["bg1"].reshape(128, 1),
        br1=P["br1"].reshape(128, 1), br2=br2p,
        coef=coef, iota8=iota8,
        idf16=ident.astype(np.float16),
    )
    if has_b3:
        shared["b3r"] = _f16(P["b3"]).reshape(1, 512)
    if has_bg2:
        shared["bg2r"] = _f16(P["bg2"]).reshape(1, 512)

    in_maps = []
    for i in range(NCORES):
        xs = x[i * R:(i + 1) * R]
        xT = np.ascontiguousarray(xs.T)
        xTh = _f16(xT)
        xTl = _f16(xT - xTh.astype(np.float32))
        m = dict(shared)
        # xh: [quarter, partition, ftile, r-in-quarter]
        m["xh"] = np.ascontiguousarray(
            xTh.reshape(4, 128, 4, R // 4).transpose(2, 1, 0, 3))
        # xl: [chunk, partition, ftile, r-in-chunk]
        m["xl"] = np.ascontiguousarray(
            xTl.reshape(4, 128, NCHUNK, CHUNK).transpose(2, 1, 0, 3))
        in_maps.append(m)

    key = (has_b3, has_bg2)
    if _cache.get("key") != key:
        _cache["nc"] = _build_program(has_b3, has_bg2)
        _cache["key"] = key
    nc = _cache["nc"]
    _cache["in_maps"] = in_maps

    res = run_bass_kernel_spmd(nc, in_maps, list(range(NCORES)))
    outs = []
    for i in range(NCORES):
        o = res.results[i]["out"].astype(np.float32)   # [3,128,R]
        o = o.reshape(384, R)[:K]
        outs.append(np.ascontiguousarray(o.T))
    return np.concatenate(outs, axis=0)


if __name__ == "__main__":
    rng = np.random.default_rng(0)
    fake = {"x": rng.standard_normal((B, D), dtype=np.float32)}
    s = lambda f: 1.0 / np.sqrt(f)
    for nm, sh, fan in [("W1", (D, H), D), ("W2", (H, H), H), ("W3", (H, D), H),
                        ("Wg1", (D, H), D), ("Wg2", (H, D), H),
                        ("Wr1", (D, H), D), ("Wr2", (H, K), H)]:
        fake[nm] = rng.uniform(-s(fan), s(fan), sh).astype(np.float32)
    for nm, sh in [("b1", H), ("b2", H), ("b3", D), ("bg1", H), ("bg2", D),
                   ("br1", H), ("br2", K)]:
        fake[nm] = np.zeros(sh, np.float32)
    out = kernel(**fake)
    print("out", out.shape, out.dtype, float(np.abs(out).max()))
